# revision 4
# baseline (speedup 1.0000x reference)
"""Multi-head self-attention Trainium2 kernel v2 (8 NeuronCores, SPMD).

Sharding: data-parallel over batch B=8 -> one batch element per core.

Single-core pipeline (bf16 matmuls, fp32 PSUM):
  qkvT = (x @ w_qkv)^T            q,k transposed; v natural+augmented
  sT_h[m,n] = k_h @ q_h^T         keys on partitions, queries free
  expT = exp(sT)                  scores in ~[-2,2]: no max subtraction
  outT_h = [v_h | 1]^T @ expT     ones column gives softmax denominator
  out_h = outT_h[:64] / outT_h[64]
  yT = w_proj^T @ outT + b_proj

v2 vs baseline:
  - software-pipelined supersteps: scores(c) + qk-proj(c+2) + AV(c-1)
    interleaved at matmul granularity so the PE never waits for exp
    (ACT engine) and ACT/DVE stay busy under PE's schedule
  - reciprocal broadcast via gpsimd partition_broadcast (SBUF->SBUF)
    instead of two DRAM DMA round-trips per divide
  - even heads' normalize writes oT directly (DVE); odd heads need the
    partition shift 0:64 -> 64:128, via one small SBUF->SBUF DMA
  - PSUM: 2x[128,1024] scores + 2x[128,512] qkv/proj + 2x[65,512] AV = 8 banks
  - q/k chunk tiles and exp tiles ring-pooled with lifetimes matched to
    the superstep schedule (fits 28MB SBUF with ~12KB/partition slack)
"""

from contextlib import ExitStack

import numpy as np
import ml_dtypes

import concourse.bass as bass
import concourse.mybir as mybir
import concourse.tile as tile
from concourse import bacc

BF16 = mybir.dt.bfloat16
F32 = mybir.dt.float32
P = 128  # SBUF partitions


def build_module(N=1024, D=1024, H=16, DK=64, reps=1):
    KC = D // P           # feature chunks (8)
    MC = N // P           # token chunks (8)
    FREE = 512            # moving free-dim per matmul (one PSUM bank fp32)
    NF = N // FREE        # 2
    assert H == 2 * KC

    nc = bacc.Bacc("TRN2", target_bir_lowering=False, debug=False)

    xT_d = nc.dram_tensor("xT", [D, N], BF16, kind="ExternalInput").ap()
    wq_d = nc.dram_tensor("wq", [D, D], BF16, kind="ExternalInput").ap()
    wk_d = nc.dram_tensor("wk", [D, D], BF16, kind="ExternalInput").ap()
    wv_d = nc.dram_tensor("wv", [D, D], BF16, kind="ExternalInput").ap()
    wp_d = nc.dram_tensor("wp", [D, D], BF16, kind="ExternalInput").ap()
    bq_d = nc.dram_tensor("bq", [P, KC], F32, kind="ExternalInput").ap()
    bk_d = nc.dram_tensor("bk", [P, KC], F32, kind="ExternalInput").ap()
    bvb_d = nc.dram_tensor("bvb", [P, D], BF16, kind="ExternalInput").ap()
    bp_d = nc.dram_tensor("bp", [P, KC], F32, kind="ExternalInput").ap()
    yT_d = nc.dram_tensor("yT", [D, N], F32, kind="ExternalOutput").ap()

    xT_v = xT_d.rearrange("(c p) n -> p c n", p=P)
    wq_v = wq_d.rearrange("(c p) n -> p c n", p=P)
    wk_v = wk_d.rearrange("(c p) n -> p c n", p=P)
    wv_v = wv_d.rearrange("(c p) n -> p c n", p=P)
    wp_v = wp_d.rearrange("(c p) n -> p c n", p=P)
    yT_v = yT_d.rearrange("(c p) n -> p c n", p=P)

    with tile.TileContext(nc) as tc, ExitStack() as ctx:
        consts = ctx.enter_context(tc.tile_pool(name="consts", bufs=1))
        perst = ctx.enter_context(tc.tile_pool(name="perst", bufs=1))
        psS = ctx.enter_context(tc.tile_pool(name="psS", bufs=2, space="PSUM"))
        psQ = ctx.enter_context(tc.tile_pool(name="psQ", bufs=2, space="PSUM"))
        psPO = ctx.enter_context(tc.tile_pool(name="psPO", bufs=2, space="PSUM"))
        qp = ctx.enter_context(tc.tile_pool(name="qp", bufs=4))
        kp = ctx.enter_context(tc.tile_pool(name="kp", bufs=4))
        exA_p = ctx.enter_context(tc.tile_pool(name="exA", bufs=12))
        exB_p = ctx.enter_context(tc.tile_pool(name="exB", bufs=14))
        misc_p = ctx.enter_context(tc.tile_pool(name="misc", bufs=2))
        yst_p = ctx.enter_context(tc.tile_pool(name="ystp", bufs=2))

        wq_sb = consts.tile([P, KC, D], BF16)
        wk_sb = consts.tile([P, KC, D], BF16)
        wv_sb = consts.tile([P, KC, D], BF16)
        wp_sb = consts.tile([P, KC, D], BF16)
        bq_sb = consts.tile([P, KC], F32)
        bk_sb = consts.tile([P, KC], F32)
        bp_sb = consts.tile([P, KC], F32)
        bvb_sb = consts.tile([P, D], BF16)

        xT_sb = perst.tile([P, KC, N], BF16)
        vaug_sb = perst.tile([P, MC, H, DK + 1], BF16)
        oT_sb = perst.tile([P, KC, N], BF16)
        nc.vector.memset(vaug_sb[:, :, :, DK : DK + 1], 1.0)

        for _rep in range(reps):
            # ---- input DMA (weights once; x re-loaded per rep) ----
            for c in range(KC):
                nc.sync.dma_start(out=xT_sb[:, c, :], in_=xT_v[:, c, :])
                if _rep == 0:
                    nc.sync.dma_start(out=wv_sb[:, c, :], in_=wv_v[:, c, :])
            if _rep == 0:
                nc.sync.dma_start(out=bvb_sb, in_=bvb_d)
                nc.sync.dma_start(out=bq_sb, in_=bq_d)
                nc.sync.dma_start(out=bk_sb, in_=bk_d)
                for c in range(KC):
                    nc.sync.dma_start(out=wq_sb[:, c, :], in_=wq_v[:, c, :])
                for c in range(KC):
                    nc.sync.dma_start(out=wk_sb[:, c, :], in_=wk_v[:, c, :])
                for c in range(KC):
                    nc.sync.dma_start(out=wp_sb[:, c, :], in_=wp_v[:, c, :])
                nc.sync.dma_start(out=bp_sb, in_=bp_d)

            # ---- v projection (natural layout into vaug) ----
            for m in range(MC):
                for f in range(NF):
                    ps = psQ.tile([P, FREE], F32, tag="psQ", name="ps_v")
                    for k in range(KC):
                        nc.tensor.matmul(
                            ps,
                            lhsT=xT_sb[:, k, m * P : (m + 1) * P],
                            rhs=wv_sb[:, k, f * FREE : (f + 1) * FREE],
                            start=(k == 0),
                            stop=(k == KC - 1),
                            skip_group_check=True,
                        )
                    nc.vector.tensor_add(
                        out=vaug_sb[:, m, 8 * f : 8 * (f + 1), 0:DK],
                        in0=ps,
                        in1=bvb_sb[:, f * FREE : (f + 1) * FREE],
                    )

            # ---- q/k projection (chunk tiles from ring pools) ----
            qt, kt = {}, {}

            def qk_mms(c):
                """Yield thunks: 32 matmuls + 4 drains for q,k of chunk c."""
                qt[c] = qp.tile([P, N], BF16, tag="q", name="qt")
                kt[c] = kp.tile([P, N], BF16, tag="k", name="kt")
                for dst, w_sb, b_sb in (
                    (qt[c], wq_sb, bq_sb),
                    (kt[c], wk_sb, bk_sb),
                ):
                    for f in range(NF):
                        ps = psQ.tile([P, FREE], F32, tag="psQ", name="ps_qk")
                        for k in range(KC):

                            def mm(ps=ps, w_sb=w_sb, c=c, f=f, k=k):
                                nc.tensor.matmul(
                                    ps,
                                    lhsT=w_sb[:, k, c * P : (c + 1) * P],
                                    rhs=xT_sb[:, k, f * FREE : (f + 1) * FREE],
                                    start=(k == 0),
                                    stop=(k == KC - 1),
                                    skip_group_check=True,
                                )

                            yield mm

                        def drain(ps=ps, dst=dst, b_sb=b_sb, c=c, f=f):
                            nc.vector.tensor_scalar_add(
                                out=dst[:, f * FREE : (f + 1) * FREE],
                                in0=ps,
                                scalar1=b_sb[:, c : c + 1],
                            )

                        yield drain

            def emit_qk(c):
                for th in qk_mms(c):
                    th()

            emit_qk(0)
            emit_qk(1)

            qk_stream = []
            for c in range(2, KC):
                qk_stream.extend(qk_mms(c))
            qk_stream.reverse()  # pop() takes from the front

            ex_tiles = {}

            def emit_scores(c, j):
                for hl, rows, tp, exp in (
                    (0, slice(0, DK), (0, 0), exA_p),
                    (1, slice(DK, P), (DK, 0), exB_p),
                ):
                    s = psS.tile([P, N], F32, tag="psS", name="s")
                    for f in range(NF):
                        nc.tensor.matmul(
                            s[:, f * FREE : (f + 1) * FREE],
                            lhsT=kt[c][rows, j * P : (j + 1) * P],
                            rhs=qt[c][rows, f * FREE : (f + 1) * FREE],
                            start=True,
                            stop=True,
                            tile_position=tp,
                            skip_group_check=True,
                        )
                    ex = exp.tile([P, N], BF16, tag="ex", name="ex")
                    nc.scalar.activation(
                        out=ex, in_=s, func=mybir.ActivationFunctionType.Exp
                    )
                    ex_tiles[(c, j, hl)] = ex

            av_state = {}

            def emit_av(c, j):
                """po tile tl=j//2 accumulates key chunks over two j-steps,
                then drains: reciprocal (DVE), partition broadcast (gpsimd),
                normalize-multiply (DVE), odd heads bounce via small DMA."""
                tl = j // 2
                hl, f = tl // 2, tl % 2
                h = c * 2 + hl
                if j % 2 == 0:
                    av_state[tl] = psPO.tile(
                        [DK + 1, FREE], F32, tag="psPO", name="po"
                    )
                po = av_state[tl]
                for jj in range(4 * (j % 2), 4 * (j % 2) + 4):
                    nc.tensor.matmul(
                        po,
                        lhsT=vaug_sb[:, jj, h, :],
                        rhs=ex_tiles[(c, jj, hl)][:, f * FREE : (f + 1) * FREE],
                        start=(jj == 0),
                        stop=(jj == MC - 1),
                        skip_group_check=True,
                    )
                if j % 2 == 1:
                    rc = misc_p.tile([1, FREE], F32, tag="rc", name="rc")
                    nc.vector.reciprocal(out=rc, in_=po[DK : DK + 1, :])
                    rcb = misc_p.tile([DK, FREE], F32, tag="rcb", name="rcb")
                    nc.gpsimd.partition_broadcast(rcb, rc)
                    fs = slice(f * FREE, (f + 1) * FREE)
                    if hl == 0:
                        nc.vector.tensor_mul(
                            out=oT_sb[0:DK, c, fs], in0=po[0:DK, :], in1=rcb
                        )
                    else:
                        tmpo = misc_p.tile([DK, FREE], BF16, tag="tmpo", name="tmpo")
                        nc.vector.tensor_mul(out=tmpo, in0=po[0:DK, :], in1=rcb)
                        nc.sync.dma_start(out=oT_sb[DK:P, c, fs], in_=tmpo)

            # ---- attention supersteps: scores(c) + qk(c+2) + AV(c-1) ----
            for c in range(KC):
                for j in range(MC):
                    emit_scores(c, j)
                    for _ in range(4):
                        if qk_stream:
                            qk_stream.pop()()
                    if c > 0:
                        emit_av(c - 1, j)
            for j in range(MC):  # AV tail for the last chunk
                emit_av(KC - 1, j)

            # ---- output projection ----
            for c in range(KC):
                for f in range(NF):
                    ps = psQ.tile([P, FREE], F32, tag="psQ", name="ps_proj")
                    for k in range(KC):
                        nc.tensor.matmul(
                            ps,
                            lhsT=wp_sb[:, k, c * P : (c + 1) * P],
                            rhs=oT_sb[:, k, f * FREE : (f + 1) * FREE],
                            start=(k == 0),
                            stop=(k == KC - 1),
                            skip_group_check=True,
                        )
                    yst = yst_p.tile([P, FREE], F32, tag="yst", name="yst")
                    nc.vector.tensor_scalar_add(
                        out=yst, in0=ps, scalar1=bp_sb[:, c : c + 1]
                    )
                    nc.sync.dma_start(
                        out=yT_v[:, c, f * FREE : (f + 1) * FREE], in_=yst
                    )

    nc.compile()
    return nc


def make_in_maps(x, w_qkv, b_qkv, w_proj, b_proj, N=1024, D=1024, H=16, DK=64):
    """Host-side prep: shard over batch, fold scale, transpose x, cast bf16."""
    bf = ml_dtypes.bfloat16
    KC = D // P
    scale = np.float32(1.0 / np.sqrt(DK))
    wq = np.ascontiguousarray((w_qkv[:, :D] * scale)).astype(bf)
    wk = np.ascontiguousarray(w_qkv[:, D : 2 * D]).astype(bf)
    wv = np.ascontiguousarray(w_qkv[:, 2 * D :]).astype(bf)
    wp = np.ascontiguousarray(w_proj).astype(bf)
    bq = np.ascontiguousarray((b_qkv[:D] * scale).reshape(KC, P).T).astype(np.float32)
    bk = np.ascontiguousarray(b_qkv[D : 2 * D].reshape(KC, P).T).astype(np.float32)
    bvb = np.ascontiguousarray(np.broadcast_to(b_qkv[2 * D :], (P, D))).astype(bf)
    bp = np.ascontiguousarray(b_proj.reshape(KC, P).T).astype(np.float32)
    in_maps = []
    for b in range(x.shape[0]):
        xT = np.ascontiguousarray(x[b].T).astype(bf)
        in_maps.append(
            dict(xT=xT, wq=wq, wk=wk, wv=wv, wp=wp, bq=bq, bk=bk, bvb=bvb, bp=bp)
        )
    return in_maps


_module_cache = {}


def kernel(x, w_qkv, b_qkv, w_proj, b_proj):
    from concourse.bass_utils import run_bass_kernel_spmd

    x = np.asarray(x)
    B = x.shape[0]
    if "nc" not in _module_cache:
        _module_cache["nc"] = build_module()
    nc = _module_cache["nc"]
    in_maps = make_in_maps(
        x, np.asarray(w_qkv), np.asarray(b_qkv), np.asarray(w_proj), np.asarray(b_proj)
    )
    res = run_bass_kernel_spmd(nc, in_maps, core_ids=list(range(B)))
    out = np.stack([np.asarray(r["yT"]).T for r in res.results], axis=0)
    return np.ascontiguousarray(out.astype(np.float32))


# revision 5
# speedup vs baseline: 1.2017x; 1.2017x over previous
"""Multi-head self-attention Trainium2 kernel v2 (8 NeuronCores, SPMD).

Sharding: data-parallel over batch B=8 -> one batch element per core.

Single-core pipeline (bf16 matmuls, fp32 PSUM):
  qkvT = (x @ w_qkv)^T            q,k transposed; v natural+augmented
  sT_h[m,n] = k_h @ q_h^T         keys on partitions, queries free
  expT = exp(sT)                  scores in ~[-2,2]: no max subtraction
  outT_h = [v_h | 1]^T @ expT     ones column gives softmax denominator
  out_h = outT_h[:64] / outT_h[64]
  yT = w_proj^T @ outT + b_proj

v2 vs baseline:
  - software-pipelined supersteps: scores(c) + qk-proj(c+2) + AV(c-1)
    interleaved at matmul granularity so the PE never waits for exp
    (ACT engine) and ACT/DVE stay busy under PE's schedule
  - reciprocal broadcast via gpsimd partition_broadcast (SBUF->SBUF)
    instead of two DRAM DMA round-trips per divide
  - even heads' normalize writes oT directly (DVE); odd heads need the
    partition shift 0:64 -> 64:128, via one small SBUF->SBUF DMA
  - PSUM: 2x[128,1024] scores + 2x[128,512] qkv/proj + 2x[65,512] AV = 8 banks
  - q/k chunk tiles and exp tiles ring-pooled with lifetimes matched to
    the superstep schedule (fits 28MB SBUF with ~12KB/partition slack)
"""

from contextlib import ExitStack

import numpy as np
import ml_dtypes

import concourse.bass as bass
import concourse.mybir as mybir
import concourse.tile as tile
from concourse import bacc

BF16 = mybir.dt.bfloat16
F32 = mybir.dt.float32
P = 128  # SBUF partitions


def build_module(N=1024, D=1024, H=16, DK=64, reps=1):
    KC = D // P           # feature chunks (8)
    MC = N // P           # token chunks (8)
    FREE = 512            # moving free-dim per matmul (one PSUM bank fp32)
    NF = N // FREE        # 2
    assert H == 2 * KC

    nc = bacc.Bacc("TRN2", target_bir_lowering=False, debug=False)

    xT_d = nc.dram_tensor("xT", [D, N], BF16, kind="ExternalInput").ap()
    wq_d = nc.dram_tensor("wq", [D, D], BF16, kind="ExternalInput").ap()
    wk_d = nc.dram_tensor("wk", [D, D], BF16, kind="ExternalInput").ap()
    wv_d = nc.dram_tensor("wv", [D, D], BF16, kind="ExternalInput").ap()
    wp_d = nc.dram_tensor("wp", [D, D], BF16, kind="ExternalInput").ap()
    bq_d = nc.dram_tensor("bq", [P, KC], F32, kind="ExternalInput").ap()
    bk_d = nc.dram_tensor("bk", [P, KC], F32, kind="ExternalInput").ap()
    bvb_d = nc.dram_tensor("bvb", [P, D], BF16, kind="ExternalInput").ap()
    bp_d = nc.dram_tensor("bp", [P, KC], F32, kind="ExternalInput").ap()
    yT_d = nc.dram_tensor("yT", [D, N], F32, kind="ExternalOutput").ap()

    xT_v = xT_d.rearrange("(c p) n -> p c n", p=P)
    wq_v = wq_d.rearrange("(c p) n -> p c n", p=P)
    wk_v = wk_d.rearrange("(c p) n -> p c n", p=P)
    wv_v = wv_d.rearrange("(c p) n -> p c n", p=P)
    wp_v = wp_d.rearrange("(c p) n -> p c n", p=P)
    yT_v = yT_d.rearrange("(c p) n -> p c n", p=P)

    with tile.TileContext(nc) as tc, ExitStack() as ctx:
        consts = ctx.enter_context(tc.tile_pool(name="consts", bufs=1))
        perst = ctx.enter_context(tc.tile_pool(name="perst", bufs=1))
        psS = ctx.enter_context(tc.tile_pool(name="psS", bufs=2, space="PSUM"))
        psQ = ctx.enter_context(tc.tile_pool(name="psQ", bufs=2, space="PSUM"))
        psPO = ctx.enter_context(tc.tile_pool(name="psPO", bufs=2, space="PSUM"))
        qp = ctx.enter_context(tc.tile_pool(name="qp", bufs=4))
        kp = ctx.enter_context(tc.tile_pool(name="kp", bufs=4))
        exA_p = ctx.enter_context(tc.tile_pool(name="exA", bufs=12))
        exB_p = ctx.enter_context(tc.tile_pool(name="exB", bufs=14))
        misc_p = ctx.enter_context(tc.tile_pool(name="misc", bufs=2))
        yst_p = ctx.enter_context(tc.tile_pool(name="ystp", bufs=3))

        wq_sb = consts.tile([P, KC, D], BF16)
        wk_sb = consts.tile([P, KC, D], BF16)
        wv_sb = consts.tile([P, KC, D], BF16)
        wp_sb = consts.tile([P, KC, D], BF16)
        bq_sb = consts.tile([P, KC], F32)
        bk_sb = consts.tile([P, KC], F32)
        bp_sb = consts.tile([P, KC], F32)
        bvb_sb = consts.tile([P, D], BF16)

        xT_sb = perst.tile([P, KC, N], BF16)
        vaug_sb = perst.tile([P, MC, H, DK + 1], BF16)
        oT_sb = perst.tile([P, KC, N], BF16)
        nc.vector.memset(vaug_sb[:, :, :, DK : DK + 1], 1.0)

        for _rep in range(reps):
            # ---- input DMA (weights once; x re-loaded per rep) ----
            for c in range(KC):
                nc.sync.dma_start(out=xT_sb[:, c, :], in_=xT_v[:, c, :])
                if _rep == 0:
                    nc.sync.dma_start(out=wv_sb[:, c, :], in_=wv_v[:, c, :])
            if _rep == 0:
                nc.sync.dma_start(out=bvb_sb, in_=bvb_d)
                nc.sync.dma_start(out=bq_sb, in_=bq_d)
                nc.sync.dma_start(out=bk_sb, in_=bk_d)
                for c in range(KC):
                    nc.sync.dma_start(out=wq_sb[:, c, :], in_=wq_v[:, c, :])
                for c in range(KC):
                    nc.sync.dma_start(out=wk_sb[:, c, :], in_=wk_v[:, c, :])
                for c in range(KC):
                    nc.sync.dma_start(out=wp_sb[:, c, :], in_=wp_v[:, c, :])
                nc.sync.dma_start(out=bp_sb, in_=bp_d)

            # ---- v projection (natural layout into vaug) ----
            for m in range(MC):
                for f in range(NF):
                    ps = psQ.tile([P, FREE], F32, tag="psQ", name="ps_v")
                    for k in range(KC):
                        nc.tensor.matmul(
                            ps,
                            lhsT=xT_sb[:, k, m * P : (m + 1) * P],
                            rhs=wv_sb[:, k, f * FREE : (f + 1) * FREE],
                            start=(k == 0),
                            stop=(k == KC - 1),
                            skip_group_check=True,
                        )
                    nc.vector.tensor_add(
                        out=vaug_sb[:, m, 8 * f : 8 * (f + 1), 0:DK],
                        in0=ps,
                        in1=bvb_sb[:, f * FREE : (f + 1) * FREE],
                    )

            # ---- q/k projection (chunk tiles from ring pools) ----
            qt, kt = {}, {}

            def qk_mms(c):
                """Yield thunks: 32 matmuls + 4 drains for q,k of chunk c."""
                qt[c] = qp.tile([P, N], BF16, tag="q", name="qt")
                kt[c] = kp.tile([P, N], BF16, tag="k", name="kt")
                for dst, w_sb, b_sb in (
                    (qt[c], wq_sb, bq_sb),
                    (kt[c], wk_sb, bk_sb),
                ):
                    for f in range(NF):
                        ps = psQ.tile([P, FREE], F32, tag="psQ", name="ps_qk")
                        for k in range(KC):

                            def mm(ps=ps, w_sb=w_sb, c=c, f=f, k=k):
                                nc.tensor.matmul(
                                    ps,
                                    lhsT=w_sb[:, k, c * P : (c + 1) * P],
                                    rhs=xT_sb[:, k, f * FREE : (f + 1) * FREE],
                                    start=(k == 0),
                                    stop=(k == KC - 1),
                                    skip_group_check=True,
                                )

                            yield mm

                        def drain(ps=ps, dst=dst, b_sb=b_sb, c=c, f=f):
                            nc.vector.tensor_scalar_add(
                                out=dst[:, f * FREE : (f + 1) * FREE],
                                in0=ps,
                                scalar1=b_sb[:, c : c + 1],
                            )

                        yield drain

            def emit_qk(c):
                for th in qk_mms(c):
                    th()

            emit_qk(0)
            emit_qk(1)

            qk_stream = []
            for c in range(2, KC):
                qk_stream.extend(qk_mms(c))
            qk_stream.reverse()  # pop() takes from the front

            ex_tiles = {}

            def emit_scores(c, j):
                for hl, rows, tp, exp in (
                    (0, slice(0, DK), (0, 0), exA_p),
                    (1, slice(DK, P), (DK, 0), exB_p),
                ):
                    s = psS.tile([P, N], F32, tag="psS", name="s")
                    for f in range(NF):
                        nc.tensor.matmul(
                            s[:, f * FREE : (f + 1) * FREE],
                            lhsT=kt[c][rows, j * P : (j + 1) * P],
                            rhs=qt[c][rows, f * FREE : (f + 1) * FREE],
                            start=True,
                            stop=True,
                            tile_position=tp,
                            skip_group_check=True,
                        )
                    ex = exp.tile([P, N], BF16, tag="ex", name="ex")
                    nc.scalar.activation(
                        out=ex, in_=s, func=mybir.ActivationFunctionType.Exp
                    )
                    ex_tiles[(c, j, hl)] = ex

            av_state = {}

            def emit_av(c, j):
                """po tile tl=j//2 accumulates key chunks over two j-steps,
                then drains: reciprocal (DVE), partition broadcast (gpsimd),
                normalize-multiply (DVE), odd heads bounce via small DMA."""
                tl = j // 2
                hl, f = tl // 2, tl % 2
                h = c * 2 + hl
                if j % 2 == 0:
                    av_state[tl] = psPO.tile(
                        [DK + 1, FREE], F32, tag="psPO", name="po"
                    )
                po = av_state[tl]
                for jj in range(4 * (j % 2), 4 * (j % 2) + 4):
                    nc.tensor.matmul(
                        po,
                        lhsT=vaug_sb[:, jj, h, :],
                        rhs=ex_tiles[(c, jj, hl)][:, f * FREE : (f + 1) * FREE],
                        start=(jj == 0),
                        stop=(jj == MC - 1),
                        skip_group_check=True,
                    )
                if j % 2 == 1:
                    rc = misc_p.tile([1, FREE], F32, tag="rc", name="rc")
                    nc.vector.reciprocal(out=rc, in_=po[DK : DK + 1, :])
                    rcb = misc_p.tile([DK, FREE], F32, tag="rcb", name="rcb")
                    nc.gpsimd.partition_broadcast(rcb, rc)
                    fs = slice(f * FREE, (f + 1) * FREE)
                    if hl == 0:
                        nc.vector.tensor_mul(
                            out=oT_sb[0:DK, c, fs], in0=po[0:DK, :], in1=rcb
                        )
                    else:
                        tmpo = misc_p.tile([DK, FREE], BF16, tag="tmpo", name="tmpo")
                        nc.vector.tensor_mul(out=tmpo, in0=po[0:DK, :], in1=rcb)
                        nc.sync.dma_start(out=oT_sb[DK:P, c, fs], in_=tmpo)

            # ---- attention supersteps: scores(c) + qk(c+2) + AV(c-1) ----
            for c in range(KC):
                for j in range(MC):
                    emit_scores(c, j)
                    for _ in range(4):
                        if qk_stream:
                            qk_stream.pop()()
                    if c > 0:
                        emit_av(c - 1, j)
            for j in range(MC):  # AV tail for the last chunk
                emit_av(KC - 1, j)

            # ---- output projection ----
            # proj(c) runs both f-halves through one 2-bank tile from the
            # scores pool (idle now; same-tag ring keeps WAR tracking on the
            # proven same-pool path) -> drain-ring stalls vanish without
            # touching the 8-bank PSUM budget
            for c in range(KC):
                ps = psS.tile([P, N], F32, tag="psS", name="ps_proj")
                for f in range(NF):
                    for k in range(KC):
                        nc.tensor.matmul(
                            ps[:, f * FREE : (f + 1) * FREE],
                            lhsT=wp_sb[:, k, c * P : (c + 1) * P],
                            rhs=oT_sb[:, k, f * FREE : (f + 1) * FREE],
                            start=(k == 0),
                            stop=(k == KC - 1),
                            skip_group_check=True,
                        )
                for f in range(NF):
                    yst = yst_p.tile([P, FREE], F32, tag="yst", name="yst")
                    nc.vector.tensor_scalar_add(
                        out=yst,
                        in0=ps[:, f * FREE : (f + 1) * FREE],
                        scalar1=bp_sb[:, c : c + 1],
                    )
                    nc.sync.dma_start(
                        out=yT_v[:, c, f * FREE : (f + 1) * FREE], in_=yst
                    )

    nc.compile()
    return nc


def make_in_maps(x, w_qkv, b_qkv, w_proj, b_proj, N=1024, D=1024, H=16, DK=64):
    """Host-side prep: shard over batch, fold scale, transpose x, cast bf16."""
    bf = ml_dtypes.bfloat16
    KC = D // P
    scale = np.float32(1.0 / np.sqrt(DK))
    wq = np.ascontiguousarray((w_qkv[:, :D] * scale)).astype(bf)
    wk = np.ascontiguousarray(w_qkv[:, D : 2 * D]).astype(bf)
    wv = np.ascontiguousarray(w_qkv[:, 2 * D :]).astype(bf)
    wp = np.ascontiguousarray(w_proj).astype(bf)
    bq = np.ascontiguousarray((b_qkv[:D] * scale).reshape(KC, P).T).astype(np.float32)
    bk = np.ascontiguousarray(b_qkv[D : 2 * D].reshape(KC, P).T).astype(np.float32)
    bvb = np.ascontiguousarray(np.broadcast_to(b_qkv[2 * D :], (P, D))).astype(bf)
    bp = np.ascontiguousarray(b_proj.reshape(KC, P).T).astype(np.float32)
    in_maps = []
    for b in range(x.shape[0]):
        xT = np.ascontiguousarray(x[b].T).astype(bf)
        in_maps.append(
            dict(xT=xT, wq=wq, wk=wk, wv=wv, wp=wp, bq=bq, bk=bk, bvb=bvb, bp=bp)
        )
    return in_maps


_module_cache = {}


def kernel(x, w_qkv, b_qkv, w_proj, b_proj):
    from concourse.bass_utils import run_bass_kernel_spmd

    x = np.asarray(x)
    B = x.shape[0]
    if "nc" not in _module_cache:
        _module_cache["nc"] = build_module()
    nc = _module_cache["nc"]
    in_maps = make_in_maps(
        x, np.asarray(w_qkv), np.asarray(b_qkv), np.asarray(w_proj), np.asarray(b_proj)
    )
    res = run_bass_kernel_spmd(nc, in_maps, core_ids=list(range(B)))
    out = np.stack([np.asarray(r["yT"]).T for r in res.results], axis=0)
    return np.ascontiguousarray(out.astype(np.float32))
